# revision 8
# baseline (speedup 1.0000x reference)
"""Correlation1D Trainium2 Bass kernel.

out[b, d, h, w] = (1/C) * sum_c in1[b, c, h, w] * in2pad[b, c, h, w + d]
  B=8, C=256, H=96, W=192, PAD=40, D=81 displacement channels.

Strategy (data-parallel over batch, 1 sample per NeuronCore):
  For each h row and each w-chunk of 96, compute a Gram band
      G[w, w'] = sum_c in1[c, w] * in2[c, w' - PAD]      (PE matmuls, k=c)
  over the valid w' range.  The needed outputs are the 81 diagonals
  O[d, w] = G[w, w + d].  Diagonals can't be extracted by any on-chip AP
  (partition/free strides are independent), so the band is written to
  DRAM scratch where a *flat* strided access pattern CAN walk diagonals:
  a gather DMA with partition stride (row_stride + 1) yields
  T[w, d] = G[w, w + d].  A PE transpose then gives O[d, w] tiles which
  are written out in the final [d, h, w] layout.  Zero-pad columns are
  materialized once in scratch instead of in SBUF so matmuls only wait
  on the input DMAs (walrus limits sync-waits per Matmult).
"""

import os

import numpy as np

import bass_rust as _br
import concourse.bass as bass
import concourse.tile as tile
from concourse import bacc
from concourse import mybir
from concourse.bass_utils import run_bass_kernel_spmd

# Problem constants (hardcoded per harness contract)
B = 8
C = 256
H = 96
W = 192
PAD = 40
D = 2 * PAD + 1  # 81
CH = 2  # c is split into CH partition-halves of 128
CP = C // CH  # 128
CHUNK = 96  # w-chunk (Gram output partition dim)
NCK = W // CHUNK  # 2
BANDW = CHUNK + D - 1  # 176  (w' window width per chunk)
VALIDW = BANDW - PAD  # 136  (valid band columns per chunk)

# Tunables (env-overridable for experiments)
HB = int(os.environ.get("CORR_HB", "4"))  # h rows per batch
NB = H // HB
BAND_DT_S = os.environ.get("CORR_BAND_DT", "fp32")  # fp32 | fp16 | bf16
MM_DT_S = os.environ.get("CORR_MM", "fp32")  # fp32 | fp32r
IN_BUFS = int(os.environ.get("CORR_IN_BUFS", "3"))
G_BUFS = int(os.environ.get("CORR_G_BUFS", "4"))

_DT = {
    "fp32": mybir.dt.float32,
    "fp16": mybir.dt.float16,
    "bf16": mybir.dt.bfloat16,
}


def _build():
    band_dt = _DT[BAND_DT_S]
    f32 = mybir.dt.float32

    nc = bacc.Bacc("TRN2")

    in1 = nc.dram_tensor("input1", [C, H, W], f32, kind="ExternalInput")
    in2 = nc.dram_tensor("input2", [C, H, W], f32, kind="ExternalInput")
    out = nc.dram_tensor("out", [D, H, W], f32, kind="ExternalOutput")
    scratch = nc.dram_tensor("scratch", [NCK, CHUNK, H, BANDW], band_dt)

    # [c, h, w] -> [p, a, h*w] so each load is one 3-dim DMA
    in1_r = in1.ap().rearrange("(a p) h w -> p a (h w)", p=CP)
    in2_r = in2.ap().rearrange("(a p) h w -> p a (h w)", p=CP)
    out_ap = out.ap()
    scr_ap = scratch.ap()

    # Valid band columns per chunk: chunk ck covers w' in
    # [ck*CHUNK, ck*CHUNK + BANDW); valid w' is [PAD, PAD + W).
    # In g-tile coords (g col = w' - PAD, 0..W):
    #   ck=0: g[:, 0:136]  -> band cols [40, 176)
    #   ck=1: g[:, 56:192] -> band cols [0, 136)
    g_lo = [max(0, ck * CHUNK - PAD) for ck in range(NCK)]
    band_lo = [max(0, PAD - ck * CHUNK) for ck in range(NCK)]

    with tile.TileContext(nc) as tc:
        with (
            tc.tile_pool(name="singles", bufs=1) as singles,
            tc.tile_pool(name="loads", bufs=IN_BUFS) as loads,
            tc.tile_pool(name="bands", bufs=2) as bands,
            tc.tile_pool(name="gats", bufs=2) as gats,
            tc.tile_pool(name="outs", bufs=2) as outs,
            tc.tile_pool(name="psg", bufs=G_BUFS, space="PSUM") as psg,
            tc.tile_pool(name="pso", bufs=2, space="PSUM") as pso,
        ):
            # identity for PE transposes
            ident = singles.tile([CHUNK, CHUNK], band_dt)
            from concourse.masks import make_identity

            make_identity(nc, ident[:])

            # pre-zero the never-written scratch pad columns:
            #   ck=0: band cols [0, 40);  ck=1: band cols [136, 176)
            zt = singles.tile([CHUNK, PAD], band_dt)
            nc.gpsimd.memset(zt[:], 0.0)
            prezero_dmas = []
            for ck, (jlo, jhi) in enumerate([(0, PAD), (VALIDW, BANDW)]):
                zsrc = bass.AP(
                    tensor=zt[:].tensor,
                    offset=zt[:].offset,
                    ap=[zt[:].ap[0], [0, H], [1, jhi - jlo]],
                )
                zd = nc.sync.dma_start(
                    out=scr_ap[ck, :, :, jlo:jhi], in_=zsrc
                )
                prezero_dmas.append(zd)

            for ib in range(NB):
                h0 = ib * HB

                in1_t = loads.tile([CP, CH, HB, W], f32)
                d1 = nc.sync.dma_start(
                    out=in1_t[:].rearrange("p a h w -> p a (h w)"),
                    in_=in1_r[:, :, h0 * W : (h0 + HB) * W],
                )
                in2_t = loads.tile([CP, CH, HB, W], f32)
                d2 = nc.sync.dma_start(
                    out=in2_t[:].rearrange("p a h w -> p a (h w)"),
                    in_=in2_r[:, :, h0 * W : (h0 + HB) * W],
                )

                band_ts = [
                    bands.tile(
                        [CHUNK, HB, VALIDW], band_dt,
                        name=f"band{ck}_{ib}", tag=f"band{ck}",
                    )
                    for ck in range(NCK)
                ]

                for hl in range(HB):
                    for ck in range(NCK):
                        g = psg.tile([CHUNK, W], f32)
                        for a in range(CH):
                            lhsT = in1_t[:, a, hl, ck * CHUNK : (ck + 1) * CHUNK]
                            rhs = in2_t[:, a, hl, :]
                            if MM_DT_S == "fp32r":
                                lhsT = lhsT.bitcast(mybir.dt.float32r)
                                rhs = rhs.bitcast(mybir.dt.float32r)
                            nc.tensor.matmul(
                                g[:],
                                lhsT,
                                rhs,
                                start=(a == 0),
                                stop=(a == CH - 1),
                            )
                        # extract valid band cols + 1/C scale (+ cast)
                        nc.scalar.mul(
                            out=band_ts[ck][:, hl, :],
                            in_=g[:, g_lo[ck] : g_lo[ck] + VALIDW],
                            mul=1.0 / C,
                        )

                band_dmas = []
                for ck in range(NCK):
                    di = nc.sync.dma_start(
                        out=scr_ap[
                            ck, :, h0 : h0 + HB,
                            band_lo[ck] : band_lo[ck] + VALIDW,
                        ],
                        in_=band_ts[ck][:],
                    )
                    band_dmas.append(di)

                # --- phase 2: skewed gather + transpose + writeout ---
                gat_ts = []
                for ck in range(NCK):
                    gat = gats.tile(
                        [CHUNK, HB, D], band_dt,
                        name=f"gat{ck}_{ib}", tag=f"gat{ck}",
                    )
                    skew = bass.AP(
                        tensor=scr_ap.tensor,
                        offset=ck * (CHUNK * H * BANDW) + h0 * BANDW,
                        ap=[[H * BANDW + 1, CHUNK], [BANDW, HB], [1, D]],
                    )
                    gi = nc.sync.dma_start(out=gat[:], in_=skew)
                    # Explicit RAW edges through DRAM scratch (belt & braces
                    # in case AP-overlap detection misses the skewed stride).
                    _br.add_dep_helper(
                        gi.ins, band_dmas[ck].ins, reason="scratch RAW"
                    )
                    _br.add_dep_helper(
                        gi.ins, prezero_dmas[ck].ins, reason="scratch prezero RAW"
                    )
                    gat_ts.append(gat)

                out_t = outs.tile([D, HB, W], f32)
                for hl in range(HB):
                    po = pso.tile([D, W], f32)
                    for ck in range(NCK):
                        nc.tensor.transpose(
                            out=po[:, ck * CHUNK : (ck + 1) * CHUNK],
                            in_=gat_ts[ck][:, hl, :],
                            identity=ident[:],
                        )
                    nc.vector.tensor_copy(out=out_t[:, hl, :], in_=po[:])
                nc.sync.dma_start(out=out_ap[:, h0 : h0 + HB, :], in_=out_t[:])

    nc.compile()
    return nc


_NC_CACHE = None


def run(input1, input2, trace=False, **spmd_kwargs):
    """Run on 8 NeuronCores; returns (out [B,D,H,W] fp32, BassKernelResults)."""
    global _NC_CACHE
    if _NC_CACHE is None:
        _NC_CACHE = _build()
    nc = _NC_CACHE

    input1 = np.ascontiguousarray(np.asarray(input1), dtype=np.float32)
    input2 = np.ascontiguousarray(np.asarray(input2), dtype=np.float32)
    assert input1.shape == (B, C, H, W) and input2.shape == (B, C, H, W)

    in_maps = [
        {"input1": input1[b], "input2": input2[b]} for b in range(B)
    ]
    res = run_bass_kernel_spmd(
        nc, in_maps, core_ids=list(range(B)), trace=trace, **spmd_kwargs
    )
    out = np.stack([res.results[b]["out"] for b in range(B)], axis=0)
    return out, res


def kernel(input1, input2):
    out, _ = run(input1, input2)
    return out


# revision 9
# speedup vs baseline: 49.8956x; 49.8956x over previous
"""Correlation1D Trainium2 Bass kernel.

out[b, d, h, w] = (1/C) * sum_c in1[b, c, h, w] * in2pad[b, c, h, w + d]
  B=8, C=256, H=96, W=192, PAD=40, D=81 displacement channels.

Strategy (data-parallel over batch, 1 sample per NeuronCore):
  For each h row and each w-chunk of 96, compute a Gram band
      G[w, w'] = sum_c in1[c, w] * in2[c, w' - PAD]      (PE matmuls, k=c)
  over the valid w' range.  The needed outputs are the 81 diagonals
  O[d, w] = G[w, w + d].  Diagonals can't be extracted by any on-chip AP
  (partition/free strides are independent), so the band is written to
  DRAM scratch where a *flat* strided access pattern CAN walk diagonals:
  a gather DMA with partition stride (row_stride + 1) yields
  T[w, d] = G[w, w + d].  A PE transpose then gives O[d, w] tiles which
  are written out in the final [d, h, w] layout.  Zero-pad columns are
  materialized once in scratch instead of in SBUF so matmuls only wait
  on the input DMAs (walrus limits sync-waits per Matmult).
"""

import os

import numpy as np

import bass_rust as _br
import concourse.bass as bass
import concourse.tile as tile
from concourse import bacc
from concourse import mybir
from concourse.bass_utils import run_bass_kernel_spmd

# Problem constants (hardcoded per harness contract)
B = 8
C = 256
H = 96
W = 192
PAD = 40
D = 2 * PAD + 1  # 81
CH = 2  # c is split into CH partition-halves of 128
CP = C // CH  # 128
CHUNK = 96  # w-chunk (Gram output partition dim)
NCK = W // CHUNK  # 2
BANDW = CHUNK + D - 1  # 176  (w' window width per chunk)
VALIDW = BANDW - PAD  # 136  (valid band columns per chunk)

# Tunables (env-overridable for experiments)
HB = int(os.environ.get("CORR_HB", "4"))  # h rows per batch
NB = H // HB
BAND_DT_S = os.environ.get("CORR_BAND_DT", "fp32")  # fp32 | fp16 | bf16
MM_DT_S = os.environ.get("CORR_MM", "fp32")  # fp32 | fp32r
IN_BUFS = int(os.environ.get("CORR_IN_BUFS", "3"))
G_BUFS = int(os.environ.get("CORR_G_BUFS", "4"))

_DT = {
    "fp32": mybir.dt.float32,
    "fp16": mybir.dt.float16,
    "bf16": mybir.dt.bfloat16,
}


def _build(reps=1):
    band_dt = _DT[BAND_DT_S]
    f32 = mybir.dt.float32

    nc = bacc.Bacc("TRN2")

    in1 = nc.dram_tensor("input1", [C, H, W], f32, kind="ExternalInput")
    in2 = nc.dram_tensor("input2", [C, H, W], f32, kind="ExternalInput")
    out = nc.dram_tensor("out", [D, H, W], f32, kind="ExternalOutput")
    scratch = nc.dram_tensor("scratch", [NCK, CHUNK, H, BANDW], band_dt)

    # [c, h, w] -> [p, a, h*w] so each load is one 3-dim DMA
    in1_r = in1.ap().rearrange("(a p) h w -> p a (h w)", p=CP)
    in2_r = in2.ap().rearrange("(a p) h w -> p a (h w)", p=CP)
    out_ap = out.ap()
    scr_ap = scratch.ap()

    # Valid band columns per chunk: chunk ck covers w' in
    # [ck*CHUNK, ck*CHUNK + BANDW); valid w' is [PAD, PAD + W).
    # In g-tile coords (g col = w' - PAD, 0..W):
    #   ck=0: g[:, 0:136]  -> band cols [40, 176)
    #   ck=1: g[:, 56:192] -> band cols [0, 136)
    g_lo = [max(0, ck * CHUNK - PAD) for ck in range(NCK)]
    band_lo = [max(0, PAD - ck * CHUNK) for ck in range(NCK)]

    with tile.TileContext(nc) as tc:
        with (
            tc.tile_pool(name="singles", bufs=1) as singles,
            tc.tile_pool(name="loads", bufs=IN_BUFS) as loads,
            tc.tile_pool(name="bands", bufs=2) as bands,
            tc.tile_pool(name="gats", bufs=2) as gats,
            tc.tile_pool(name="outs", bufs=2) as outs,
            tc.tile_pool(name="psg", bufs=G_BUFS, space="PSUM") as psg,
            tc.tile_pool(name="pso", bufs=2, space="PSUM") as pso,
        ):
            # identity for PE transposes
            ident = singles.tile([CHUNK, CHUNK], band_dt)
            from concourse.masks import make_identity

            make_identity(nc, ident[:])

            # pre-zero the never-written scratch pad columns:
            #   ck=0: band cols [0, 40);  ck=1: band cols [136, 176)
            zt = singles.tile([CHUNK, PAD], band_dt)
            nc.gpsimd.memset(zt[:], 0.0)
            prezero_dmas = []
            for ck, (jlo, jhi) in enumerate([(0, PAD), (VALIDW, BANDW)]):
                zsrc = bass.AP(
                    tensor=zt[:].tensor,
                    offset=zt[:].offset,
                    ap=[zt[:].ap[0], [0, H], [1, jhi - jlo]],
                )
                zd = nc.sync.dma_start(
                    out=scr_ap[ck, :, :, jlo:jhi], in_=zsrc
                )
                prezero_dmas.append(zd)

            for _rep in range(reps):
              for ib in range(NB):
                h0 = ib * HB

                in1_t = loads.tile([CP, CH, HB, W], f32)
                d1 = nc.sync.dma_start(
                    out=in1_t[:].rearrange("p a h w -> p a (h w)"),
                    in_=in1_r[:, :, h0 * W : (h0 + HB) * W],
                )
                in2_t = loads.tile([CP, CH, HB, W], f32)
                d2 = nc.sync.dma_start(
                    out=in2_t[:].rearrange("p a h w -> p a (h w)"),
                    in_=in2_r[:, :, h0 * W : (h0 + HB) * W],
                )

                band_ts = [
                    bands.tile(
                        [CHUNK, HB, VALIDW], band_dt,
                        name=f"band{ck}_{_rep}_{ib}", tag=f"band{ck}",
                    )
                    for ck in range(NCK)
                ]

                for hl in range(HB):
                    for ck in range(NCK):
                        g = psg.tile([CHUNK, W], f32)
                        for a in range(CH):
                            lhsT = in1_t[:, a, hl, ck * CHUNK : (ck + 1) * CHUNK]
                            rhs = in2_t[:, a, hl, :]
                            if MM_DT_S == "fp32r":
                                lhsT = lhsT.bitcast(mybir.dt.float32r)
                                rhs = rhs.bitcast(mybir.dt.float32r)
                            nc.tensor.matmul(
                                g[:],
                                lhsT,
                                rhs,
                                start=(a == 0),
                                stop=(a == CH - 1),
                            )
                        # extract valid band cols + 1/C scale (+ cast)
                        nc.scalar.mul(
                            out=band_ts[ck][:, hl, :],
                            in_=g[:, g_lo[ck] : g_lo[ck] + VALIDW],
                            mul=1.0 / C,
                        )

                band_dmas = []
                for ck in range(NCK):
                    di = nc.sync.dma_start(
                        out=scr_ap[
                            ck, :, h0 : h0 + HB,
                            band_lo[ck] : band_lo[ck] + VALIDW,
                        ],
                        in_=band_ts[ck][:],
                    )
                    band_dmas.append(di)

                # --- phase 2: skewed gather + transpose + writeout ---
                gat_ts = []
                for ck in range(NCK):
                    gat = gats.tile(
                        [CHUNK, HB, D], band_dt,
                        name=f"gat{ck}_{_rep}_{ib}", tag=f"gat{ck}",
                    )
                    skew = bass.AP(
                        tensor=scr_ap.tensor,
                        offset=ck * (CHUNK * H * BANDW) + h0 * BANDW,
                        ap=[[H * BANDW + 1, CHUNK], [BANDW, HB], [1, D]],
                    )
                    gi = nc.sync.dma_start(out=gat[:], in_=skew)
                    # Explicit RAW edges through DRAM scratch (belt & braces
                    # in case AP-overlap detection misses the skewed stride).
                    _br.add_dep_helper(
                        gi.ins, band_dmas[ck].ins, reason="scratch RAW"
                    )
                    _br.add_dep_helper(
                        gi.ins, prezero_dmas[ck].ins, reason="scratch prezero RAW"
                    )
                    gat_ts.append(gat)

                out_t = outs.tile([D, HB, W], f32)
                for hl in range(HB):
                    po = pso.tile([D, W], f32)
                    for ck in range(NCK):
                        nc.tensor.transpose(
                            out=po[:, ck * CHUNK : (ck + 1) * CHUNK],
                            in_=gat_ts[ck][:, hl, :],
                            identity=ident[:],
                        )
                    nc.vector.tensor_copy(out=out_t[:, hl, :], in_=po[:])
                nc.sync.dma_start(out=out_ap[:, h0 : h0 + HB, :], in_=out_t[:])

    nc.compile()
    return nc


_NC_CACHE = None


def run(input1, input2, trace=False, **spmd_kwargs):
    """Run on 8 NeuronCores; returns (out [B,D,H,W] fp32, BassKernelResults)."""
    global _NC_CACHE
    if _NC_CACHE is None:
        _NC_CACHE = _build()
    nc = _NC_CACHE

    input1 = np.ascontiguousarray(np.asarray(input1), dtype=np.float32)
    input2 = np.ascontiguousarray(np.asarray(input2), dtype=np.float32)
    assert input1.shape == (B, C, H, W) and input2.shape == (B, C, H, W)

    in_maps = [
        {"input1": input1[b], "input2": input2[b]} for b in range(B)
    ]
    res = run_bass_kernel_spmd(
        nc, in_maps, core_ids=list(range(B)), trace=trace, **spmd_kwargs
    )
    out = np.stack([res.results[b]["out"] for b in range(B)], axis=0)
    return out, res


def kernel(input1, input2):
    out, _ = run(input1, input2)
    return out
